# revision 27
# baseline (speedup 1.0000x reference)
"""Trainium2 Bass kernel for nn_Conv2dShareQ (vq_codebook) — Winograd F(2x2,3x3).

Computation (see reference):
    wq = centroids[labels]            # [512, 256, 3, 3] fp32, 16-entry codebook
    out0 = conv2d(x, wq[:256], bias[0])   # NCHW, 3x3, stride 1, pad 1
    out1 = conv2d(x, wq[256:], bias[1])

Sharding: 8-way data-parallel over batch; each core runs 2 images through BOTH
convs (512 out channels = 4 m-tiles) so the transformed input V is shared.

Winograd F(2x2,3x3) cuts PE work 2.25x vs direct conv:
    Y = A^T [ (G w G^T) . (B^T d B) ] A   per 4x4 input tile, stride 2.
Host precomputes U = G wq G^T (a=3 slice negated so the device row stage is a
pure add) in matmul lhsT layout, plus zero-padded bf16 x.  Per (image, half):
  - input transform on DVE: 4+16 strided tensor_tensor ops -> V[16pt][128,392]
  - 8 matmuls per (mt, b): M_a = sum_kt U^T V into one 4-bank PSUM tile
  - row stage: ACT evicts M1,M2 (one op); GpSimd forms (M1+M2, M1-M2);
    DVE adds (M0, -M3) from PSUM -> S pair (bf16)
  - col stage on GpSimd: A-combos + bias via scalar_tensor_tensor, written
    strided into a row-major output tile; DMA out bf16 (host upcasts).
"""

import sys

for _p in ("/opt/trn_rl_repo", "/root/.axon_site/_ro/trn_rl_repo"):
    if _p not in sys.path:
        sys.path.append(_p)

import numpy as np
import ml_dtypes

import concourse.bass as bass
import concourse.mybir as mybir
from concourse.tile import TileContext, ScopedClock
from concourse.tile_scheduler import N_PROCS
from bass_rust import VectorClock
from concourse.bass_utils import run_bass_kernel_spmd

F32 = mybir.dt.float32
BF16 = mybir.dt.bfloat16
ADD = mybir.AluOpType.add
SUB = mybir.AluOpType.subtract
IDENT = mybir.ActivationFunctionType.Identity

N_IMG = 2               # 16 images / 8 cores
N_KT = 2                # 256 input channels / 128
N_MT = 4                # 512 output channels / 128 (both conv groups)
N_PT = 16               # winograd transform points
H = W = 56
HP = 58                 # padded
HWP = HP * HP
HW = H * W
NTH = 14                # winograd tile rows per h-half
FH = NTH * 28           # 392 free elems per (point, half)
N_WARM = 40


class SplitDrainTileContext(TileContext):
    """Tail drain split one proc per drain: this walrus build rejects CTRL
    instructions carrying more than one sem wait."""

    def _drain_and_barrier(self, tick_clock, wait_clock):
        gc = tick_clock.global_clock
        for p in range(N_PROCS):
            t = gc[p]
            if t <= 0:
                continue
            vec = [t if q == p else 0 for q in range(N_PROCS)]
            d = self.nc.sync.drain()
            wait_clock.add_sem_waits(d.ins, ScopedClock({None: VectorClock(vec)}))
        self.nc.all_engine_barrier()
        assert self.sems is not None
        popped = self.nc._tile_sem_poison_stack.pop()
        assert popped is self._sem_poison
        self.nc.clear_and_free_semaphores(list(self.sems.allocated().values()))
        self.nc.all_engine_barrier()


def _split_multi_waits(nc, limit=1):
    """This walrus build rejects instructions carrying more than one sem wait
    ("Too many sync wait commands").  Hoist excess waits onto wait-only
    EventSemaphore instructions inserted just before, on the same engine."""
    for f in nc.m.functions:
        for bb in f.blocks:
            out = []
            for ins in bb.instructions:
                si = ins.sync_info
                if si is not None and si.on_wait and len(si.on_wait) > limit:
                    waits = list(si.on_wait)
                    for w in waits[:-limit]:
                        es = mybir.InstEventSemaphore(
                            name=f"waitsplit_{nc.next_id()}", ins=[], outs=[])
                        es.engine = ins.engine
                        es.sync_info = mybir.SyncInfo(on_wait=[w], on_update=[])
                        out.append(es)
                    si.on_wait = waits[-limit:]
                out.append(ins)
            bb.instructions[:] = out


def build_program():
    nc = bass.Bass()

    x_in = nc.dram_tensor("x", [N_IMG, N_KT, 128, HWP], BF16,
                          kind="ExternalInput")
    u_in = nc.dram_tensor("u", [N_KT, 128, N_MT * N_PT * 128], BF16,
                          kind="ExternalInput")
    bias_in = nc.dram_tensor("bias", [N_MT, 128], F32, kind="ExternalInput")
    out = nc.dram_tensor("out", [N_IMG, N_MT, 128, HW], BF16,
                         kind="ExternalOutput")

    with SplitDrainTileContext(nc) as tc:
        with (
            tc.tile_pool(name="consts", bufs=1) as consts,
            tc.tile_pool(name="u", bufs=1) as u_pool,
            tc.tile_pool(name="xpad", bufs=1) as xpad_pool,
            tc.tile_pool(name="tT", bufs=1) as tT_pool,
            tc.tile_pool(name="v", bufs=3) as v_pool,
            tc.tile_pool(name="e", bufs=4) as e_pool,
            tc.tile_pool(name="tg", bufs=4) as tg_pool,
            tc.tile_pool(name="s", bufs=2) as s_pool,
            tc.tile_pool(name="ob", bufs=4) as ob_pool,
            tc.tile_pool(name="psum", bufs=2, space="PSUM") as psum_pool,
        ):
            u_sb = [u_pool.tile([128, N_MT * N_PT * 128], BF16, tag=f"u{kt}",
                                name=f"u{kt}") for kt in range(N_KT)]
            xpad = [[xpad_pool.tile([128, HP, HP], BF16, tag=f"xp{im}_{kt}",
                                    name=f"xpad{im}_{kt}")
                     for kt in range(N_KT)] for im in range(N_IMG)]

            # ---- critical DMAs: mt0 weights for both kt + img0 x ----
            UC = N_PT * 128          # 2048 free elems per (kt, mt) chunk
            for kt in range(N_KT):
                nc.sync.dma_start(out=u_sb[kt][:, 0:UC], in_=u_in[kt][:, 0:UC])

            def load_x(im):
                for kt in range(N_KT):
                    xp = xpad[im][kt]
                    nc.sync.dma_start(out=xp[:, 0:30, :],
                                      in_=x_in[im, kt][:, 0:30 * HP])
                    nc.sync.dma_start(out=xp[:, 30:HP, :],
                                      in_=x_in[im, kt][:, 30 * HP:HWP])

            load_x(0)
            bias_sb = consts.tile([128, N_MT], F32)
            for mt in range(N_MT):
                nc.sync.dma_start(out=bias_sb[:, mt:mt + 1], in_=bias_in[mt, :])

            # ---- PE clock ramp on zeros while DMAs fly ----
            warm_sb = consts.tile([128, 256], BF16)
            nc.gpsimd.memset(warm_sb[:], 0.0)
            warm_ps = psum_pool.tile([128, 256], F32, tag="m")
            for _ in range(N_WARM):
                nc.tensor.matmul(warm_ps[:], warm_sb[:, :128], warm_sb[:],
                                 start=True, stop=True)

            # ---- remaining DMAs ----
            for mt in range(1, N_MT):
                for kt in range(N_KT):
                    nc.sync.dma_start(out=u_sb[kt][:, mt * UC:(mt + 1) * UC],
                                      in_=u_in[kt][:, mt * UC:(mt + 1) * UC])
            load_x(1)

            # ---- input transform: (im, h) -> V[kt] [128, 16, 14, 28] ----
            def transform(im, h):
                r0 = 28 * h
                vts = []
                for kt in range(N_KT):
                    xv = xpad[im][kt]
                    T = tT_pool.tile([128, 4, NTH, HP], BF16, tag=f"T{kt}",
                                     name=f"T{kt}")
                    nc.vector.tensor_tensor(
                        out=T[:, 0], in0=xv[:, r0 + 0:r0 + 28:2, :],
                        in1=xv[:, r0 + 2:r0 + 30:2, :], op=SUB)
                    nc.vector.tensor_tensor(
                        out=T[:, 1], in0=xv[:, r0 + 1:r0 + 29:2, :],
                        in1=xv[:, r0 + 2:r0 + 30:2, :], op=ADD)
                    nc.vector.tensor_tensor(
                        out=T[:, 2], in0=xv[:, r0 + 2:r0 + 30:2, :],
                        in1=xv[:, r0 + 1:r0 + 29:2, :], op=SUB)
                    nc.vector.tensor_tensor(
                        out=T[:, 3], in0=xv[:, r0 + 1:r0 + 29:2, :],
                        in1=xv[:, r0 + 3:min(r0 + 31, HP):2, :], op=SUB)
                    v = v_pool.tile([128, N_PT, NTH, 28], BF16, tag=f"v{kt}",
                                    name=f"v{kt}")
                    for b in range(4):
                        if b == 0:
                            c0, c1, op = 0, 2, SUB
                        elif b == 1:
                            c0, c1, op = 1, 2, ADD
                        elif b == 2:
                            c0, c1, op = 2, 1, SUB
                        else:
                            c0, c1, op = 1, 3, SUB
                        for a in range(4):
                            nc.vector.tensor_tensor(
                                out=v[:, 4 * a + b],
                                in0=T[:, a, :, c0:min(c0 + 56, HP):2],
                                in1=T[:, a, :, c1:min(c1 + 56, HP):2], op=op)
                    vts.append(v)
                return vts

            # ---- matmuls + output transform for one (im, h) ----
            def group(im, h, vts):
                for mt in range(N_MT):
                    ob = ob_pool.tile([128, NTH, 2, 28, 2], BF16, tag="ob",
                                      name="ob")
                    sp = []
                    for b in range(4):
                        M4 = psum_pool.tile([128, 4, 512], F32, tag="m",
                                            name="m")
                        for a in range(4):
                            p = 4 * a + b
                            off = (mt * N_PT + p) * 128
                            for kt in range(N_KT):
                                nc.tensor.matmul(
                                    M4[:, a, 0:FH],
                                    u_sb[kt][:, off:off + 128],
                                    vts[kt][:, p],
                                    start=(kt == 0), stop=(kt == N_KT - 1))
                        # bias enters once per output: e1 = M1 + b on the
                        # b==1 column makes t01 = M1+M2+b and t12 = M1-M2+b,
                        # and S_{a',1} appears with +1 in both Y columns
                        e12 = e_pool.tile([128, 2, FH], BF16, tag="e",
                                          name="e12")
                        b1 = bias_sb[:, mt:mt + 1] if b == 1 else 0.0
                        nc.scalar.activation(out=e12[:, 0], in_=M4[:, 1, 0:FH],
                                             func=IDENT, scale=1.0, bias=b1)
                        nc.scalar.activation(out=e12[:, 1], in_=M4[:, 2, 0:FH],
                                             func=IDENT, scale=1.0)
                        tp = tg_pool.tile([128, 2, FH], BF16, tag="tp",
                                          name="tp")
                        nc.gpsimd.tensor_tensor(out=tp[:, 0], in0=e12[:, 0],
                                                in1=e12[:, 1], op=ADD)
                        nc.gpsimd.tensor_tensor(out=tp[:, 1], in0=e12[:, 0],
                                                in1=e12[:, 1], op=SUB)
                        s2 = s_pool.tile([128, 2, NTH, 28], BF16,
                                         tag=f"s{b}", name=f"s{b}")
                        # S0 = M0 + (M1+M2); S1 = (M1-M2) + M3' (U a=3 negated)
                        nc.vector.tensor_tensor(out=s2[:],
                                                in0=M4[:, 0:4:3, 0:FH],
                                                in1=tp[:], op=ADD)
                        sp.append(s2)
                    for ap in range(2):
                        q0 = tg_pool.tile([128, NTH, 28], BF16, tag="q",
                                          name="q")
                        nc.gpsimd.tensor_tensor(out=q0[:], in0=sp[0][:, ap],
                                                in1=sp[1][:, ap], op=ADD)
                        nc.gpsimd.tensor_tensor(out=ob[:, :, ap, :, 0],
                                                in0=q0[:], in1=sp[2][:, ap],
                                                op=ADD)
                        q1 = tg_pool.tile([128, NTH, 28], BF16, tag="q",
                                          name="q")
                        nc.gpsimd.tensor_tensor(out=q1[:], in0=sp[1][:, ap],
                                                in1=sp[2][:, ap], op=SUB)
                        nc.gpsimd.tensor_tensor(out=ob[:, :, ap, :, 1],
                                                in0=q1[:], in1=sp[3][:, ap],
                                                op=SUB)
                    nc.sync.dma_start(
                        out=out[im, mt][:, 28 * h * W: 28 * h * W + 4 * FH],
                        in_=ob[:])

            v00 = transform(0, 0)
            v01 = transform(0, 1)
            group(0, 0, v00)
            v10 = transform(1, 0)
            group(0, 1, v01)
            v11 = transform(1, 1)
            group(1, 0, v10)
            group(1, 1, v11)

    _split_multi_waits(nc)
    return nc


_NC_CACHE = None


def _get_nc():
    global _NC_CACHE
    if _NC_CACHE is None:
        _NC_CACHE = build_program()
    return _NC_CACHE


_G = np.array([[1, 0, 0], [.5, .5, .5], [.5, -.5, .5], [0, 0, 1]], np.float64)


def make_in_maps(x, centroids, labels, bias):
    """Shard full inputs into 8 per-core input maps (host-side gather,
    padding, and Winograd weight transform)."""
    x = np.ascontiguousarray(x, dtype=np.float32)
    centroids = np.ascontiguousarray(centroids, dtype=np.float32)
    labels = np.ascontiguousarray(labels, dtype=np.int64)
    bias = np.ascontiguousarray(bias, dtype=np.float32)

    xp = np.zeros((16, 256, HP, HP), dtype=ml_dtypes.bfloat16)
    xp[:, :, 1:1 + H, 1:1 + W] = x
    xp = xp.reshape(16, N_KT, 128, HWP)

    wq = centroids[labels]                       # [512, 256, 3, 3] f32
    U = np.einsum("ai,ocij,bj->aboc", _G, wq.astype(np.float64), _G)
    U[3] = -U[3]                                 # device row stage is pure add
    # -> lhsT layout [kt, cc, mt, p=4a+b, oo]
    U = U.reshape(16, N_MT, 128, N_KT, 128)      # [p, mt, oo, kt, cc]
    U = U.transpose(3, 4, 1, 0, 2)               # [kt, cc, mt, p, oo]
    U = np.ascontiguousarray(U.reshape(N_KT, 128, N_MT * N_PT * 128))
    U = U.astype(ml_dtypes.bfloat16)

    bias_l = np.ascontiguousarray(
        np.concatenate([bias[0], bias[1]]).reshape(N_MT, 128))

    in_maps = []
    for c in range(8):
        in_maps.append({
            "x": np.ascontiguousarray(xp[2 * c: 2 * c + 2]),
            "u": U,
            "bias": np.ascontiguousarray(bias_l),
        })
    return in_maps


def run(x, centroids, labels, bias, trace=False, trace_cores=None):
    nc = _get_nc()
    in_maps = make_in_maps(x, centroids, labels, bias)
    res = run_bass_kernel_spmd(nc, in_maps, list(range(8)), trace=trace,
                               trace_cores=trace_cores)
    out0 = np.empty((16, 256, H, W), dtype=np.float32)
    out1 = np.empty((16, 256, H, W), dtype=np.float32)
    for c in range(8):
        o = res.results[c]["out"].reshape(N_IMG, 512, H, W).astype(np.float32)
        out0[2 * c: 2 * c + 2] = o[:, :256]
        out1[2 * c: 2 * c + 2] = o[:, 256:]
    return (out0, out1), res


def kernel(x, centroids, labels, bias):
    (out0, out1), _ = run(x, centroids, labels, bias, trace=False)
    return (out0, out1)


# revision 30
# speedup vs baseline: 1.2768x; 1.2768x over previous
"""Trainium2 Bass kernel for nn_Conv2dShareQ (vq_codebook) — Winograd F(2x2,3x3).

Computation (see reference):
    wq = centroids[labels]            # [512, 256, 3, 3] fp32, 16-entry codebook
    out0 = conv2d(x, wq[:256], bias[0])   # NCHW, 3x3, stride 1, pad 1
    out1 = conv2d(x, wq[256:], bias[1])

Sharding: 8-way data-parallel over batch; each core runs 2 images through BOTH
convs (512 out channels = 4 m-tiles) so the transformed input V is shared.

Winograd F(2x2,3x3) cuts PE work 2.25x vs direct conv:
    Y = A^T [ (G w G^T) . (B^T d B) ] A   per 4x4 input tile, stride 2.
Host precomputes U = G wq G^T (a=3 slice negated so the device row stage is a
pure add) in matmul lhsT layout, plus zero-padded bf16 x.  Per (image, half):
  - input transform on DVE: 4+16 strided tensor_tensor ops -> V[16pt][128,392]
  - 8 matmuls per (mt, b): M_a = sum_kt U^T V into one 4-bank PSUM tile
  - row stage: ACT evicts M1,M2 (one op); GpSimd forms (M1+M2, M1-M2);
    DVE adds (M0, -M3) from PSUM -> S pair (bf16)
  - col stage on GpSimd: A-combos + bias via scalar_tensor_tensor, written
    strided into a row-major output tile; DMA out bf16 (host upcasts).
"""

import sys

for _p in ("/opt/trn_rl_repo", "/root/.axon_site/_ro/trn_rl_repo"):
    if _p not in sys.path:
        sys.path.append(_p)

import numpy as np
import ml_dtypes

import concourse.bass as bass
import concourse.mybir as mybir
from concourse.tile import TileContext, ScopedClock
from concourse.tile_scheduler import N_PROCS
from bass_rust import VectorClock
from concourse.bass_utils import run_bass_kernel_spmd

F32 = mybir.dt.float32
BF16 = mybir.dt.bfloat16
ADD = mybir.AluOpType.add
SUB = mybir.AluOpType.subtract
IDENT = mybir.ActivationFunctionType.Identity

N_IMG = 2               # 16 images / 8 cores
N_KT = 2                # 256 input channels / 128
N_MT = 4                # 512 output channels / 128 (both conv groups)
N_PT = 16               # winograd transform points
H = W = 56
HP = 58                 # padded
HWP = HP * HP
HW = H * W
NTH = 14                # winograd tile rows per h-half
FH = NTH * 28           # 392 free elems per (point, half)
N_WARM = 40


class SplitDrainTileContext(TileContext):
    """Tail drain split one proc per drain: this walrus build rejects CTRL
    instructions carrying more than one sem wait."""

    def _drain_and_barrier(self, tick_clock, wait_clock):
        gc = tick_clock.global_clock
        for p in range(N_PROCS):
            t = gc[p]
            if t <= 0:
                continue
            vec = [t if q == p else 0 for q in range(N_PROCS)]
            d = self.nc.sync.drain()
            wait_clock.add_sem_waits(d.ins, ScopedClock({None: VectorClock(vec)}))
        self.nc.all_engine_barrier()
        assert self.sems is not None
        popped = self.nc._tile_sem_poison_stack.pop()
        assert popped is self._sem_poison
        self.nc.clear_and_free_semaphores(list(self.sems.allocated().values()))
        self.nc.all_engine_barrier()


def _split_multi_waits(nc, limit=1):
    """This walrus build rejects instructions carrying more than one sem wait
    ("Too many sync wait commands").  Hoist excess waits onto wait-only
    EventSemaphore instructions inserted just before, on the same engine."""
    for f in nc.m.functions:
        for bb in f.blocks:
            out = []
            for ins in bb.instructions:
                si = ins.sync_info
                if si is not None and si.on_wait and len(si.on_wait) > limit:
                    waits = list(si.on_wait)
                    for w in waits[:-limit]:
                        es = mybir.InstEventSemaphore(
                            name=f"waitsplit_{nc.next_id()}", ins=[], outs=[])
                        es.engine = ins.engine
                        es.sync_info = mybir.SyncInfo(on_wait=[w], on_update=[])
                        out.append(es)
                    si.on_wait = waits[-limit:]
                out.append(ins)
            bb.instructions[:] = out


def build_program():
    nc = bass.Bass()

    x_in = nc.dram_tensor("x", [N_IMG, N_KT, 128, HWP], BF16,
                          kind="ExternalInput")
    u_in = nc.dram_tensor("u", [N_KT, 128, N_MT * N_PT * 128], BF16,
                          kind="ExternalInput")
    bias_in = nc.dram_tensor("bias", [N_MT, 128], F32, kind="ExternalInput")
    out = nc.dram_tensor("out", [N_IMG, N_MT, 128, HW], BF16,
                         kind="ExternalOutput")

    with SplitDrainTileContext(nc) as tc:
        with (
            tc.tile_pool(name="consts", bufs=1) as consts,
            tc.tile_pool(name="u", bufs=1) as u_pool,
            tc.tile_pool(name="xpad", bufs=1) as xpad_pool,
            tc.tile_pool(name="tT", bufs=1) as tT_pool,
            tc.tile_pool(name="v", bufs=3) as v_pool,
            tc.tile_pool(name="e", bufs=4) as e_pool,
            tc.tile_pool(name="tg", bufs=4) as tg_pool,
            tc.tile_pool(name="s", bufs=2) as s_pool,
            tc.tile_pool(name="ob", bufs=4) as ob_pool,
            tc.tile_pool(name="psum", bufs=2, space="PSUM") as psum_pool,
        ):
            u_sb = [u_pool.tile([128, N_MT * N_PT * 128], BF16, tag=f"u{kt}",
                                name=f"u{kt}") for kt in range(N_KT)]
            xpad = [[xpad_pool.tile([128, HP, HP], BF16, tag=f"xp{im}_{kt}",
                                    name=f"xpad{im}_{kt}")
                     for kt in range(N_KT)] for im in range(N_IMG)]

            # ---- critical DMAs: mt0 weights for both kt + img0 x ----
            UC = N_PT * 128          # 2048 free elems per (kt, mt) chunk
            for kt in range(N_KT):
                nc.sync.dma_start(out=u_sb[kt][:, 0:UC], in_=u_in[kt][:, 0:UC])

            def load_x(im):
                for kt in range(N_KT):
                    xp = xpad[im][kt]
                    nc.sync.dma_start(out=xp[:, 0:30, :],
                                      in_=x_in[im, kt][:, 0:30 * HP])
                    nc.sync.dma_start(out=xp[:, 30:HP, :],
                                      in_=x_in[im, kt][:, 30 * HP:HWP])

            load_x(0)
            bias_sb = consts.tile([128, N_MT], F32)
            for mt in range(N_MT):
                nc.sync.dma_start(out=bias_sb[:, mt:mt + 1], in_=bias_in[mt, :])

            # ---- PE clock ramp on zeros while DMAs fly ----
            warm_sb = consts.tile([128, 256], BF16)
            nc.gpsimd.memset(warm_sb[:], 0.0)
            warm_ps = psum_pool.tile([128, 256], F32, tag="m")
            for _ in range(N_WARM):
                nc.tensor.matmul(warm_ps[:], warm_sb[:, :128], warm_sb[:],
                                 start=True, stop=True)

            # ---- remaining DMAs ----
            for mt in range(1, N_MT):
                for kt in range(N_KT):
                    nc.sync.dma_start(out=u_sb[kt][:, mt * UC:(mt + 1) * UC],
                                      in_=u_in[kt][:, mt * UC:(mt + 1) * UC])
            load_x(1)

            # ---- input transform: (im, h) -> V[kt] [128, 16, 14, 28] ----
            def transform(im, h):
                r0 = 28 * h
                vts = []
                for kt in range(N_KT):
                    xv = xpad[im][kt]
                    T = tT_pool.tile([128, 4, NTH, HP], BF16, tag=f"T{kt}",
                                     name=f"T{kt}")
                    nc.vector.tensor_tensor(
                        out=T[:, 0], in0=xv[:, r0 + 0:r0 + 28:2, :],
                        in1=xv[:, r0 + 2:r0 + 30:2, :], op=SUB)
                    nc.vector.tensor_tensor(
                        out=T[:, 1], in0=xv[:, r0 + 1:r0 + 29:2, :],
                        in1=xv[:, r0 + 2:r0 + 30:2, :], op=ADD)
                    nc.vector.tensor_tensor(
                        out=T[:, 2], in0=xv[:, r0 + 2:r0 + 30:2, :],
                        in1=xv[:, r0 + 1:r0 + 29:2, :], op=SUB)
                    nc.vector.tensor_tensor(
                        out=T[:, 3], in0=xv[:, r0 + 1:r0 + 29:2, :],
                        in1=xv[:, r0 + 3:min(r0 + 31, HP):2, :], op=SUB)
                    v = v_pool.tile([128, N_PT, NTH, 28], BF16, tag=f"v{kt}",
                                    name=f"v{kt}")
                    # fused over a: v[:, b::4] = all four row-points of col b
                    for b, (c0, c1, op) in enumerate(
                            [(0, 2, SUB), (1, 2, ADD), (2, 1, SUB),
                             (1, 3, SUB)]):
                        nc.vector.tensor_tensor(
                            out=v[:, b::4],
                            in0=T[:, :, :, c0:min(c0 + 56, HP):2],
                            in1=T[:, :, :, c1:min(c1 + 56, HP):2], op=op)
                    vts.append(v)
                return vts

            # ---- matmuls + output transform for one (im, h) ----
            def group(im, h, vts):
                for mt in range(N_MT):
                    ob = ob_pool.tile([128, NTH, 2, 28, 2], BF16, tag="ob",
                                      name="ob")
                    sp = []
                    for b in range(4):
                        M4 = psum_pool.tile([128, 4, 512], F32, tag="m",
                                            name="m")
                        for a in range(4):
                            p = 4 * a + b
                            off = (mt * N_PT + p) * 128
                            for kt in range(N_KT):
                                nc.tensor.matmul(
                                    M4[:, a, 0:FH],
                                    u_sb[kt][:, off:off + 128],
                                    vts[kt][:, p],
                                    start=(kt == 0), stop=(kt == N_KT - 1))
                        # bias enters once per output: e1 = M1 + b on the
                        # b==1 column makes t01 = M1+M2+b and t12 = M1-M2+b,
                        # and S_{a',1} appears with +1 in both Y columns
                        e12 = e_pool.tile([128, 2, FH], BF16, tag="e",
                                          name="e12")
                        if b == 1:
                            nc.scalar.activation(
                                out=e12[:, 0], in_=M4[:, 1, 0:FH], func=IDENT,
                                scale=1.0, bias=bias_sb[:, mt:mt + 1])
                            nc.scalar.activation(
                                out=e12[:, 1], in_=M4[:, 2, 0:FH], func=IDENT,
                                scale=1.0)
                        else:
                            nc.scalar.activation(
                                out=e12[:], in_=M4[:, 1:3, 0:FH], func=IDENT,
                                scale=1.0)
                        tp = tg_pool.tile([128, 2, FH], BF16, tag="tp",
                                          name="tp")
                        nc.vector.tensor_tensor(out=tp[:, 0], in0=e12[:, 0],
                                                in1=e12[:, 1], op=ADD)
                        nc.vector.tensor_tensor(out=tp[:, 1], in0=e12[:, 0],
                                                in1=e12[:, 1], op=SUB)
                        s2 = s_pool.tile([128, 2, NTH, 28], BF16,
                                         tag=f"s{b}", name=f"s{b}")
                        # S0 = M0 + (M1+M2); S1 = (M1-M2) + M3' (U a=3 negated)
                        nc.vector.tensor_tensor(out=s2[:],
                                                in0=M4[:, 0:4:3, 0:FH],
                                                in1=tp[:], op=ADD)
                        sp.append(s2)
                    # col stage fused over both output-row lanes; ob views
                    # iterate (lane, tile-row, tx) to match the S layout
                    obb = ob[:]

                    def ob_view(bp):
                        return bass.AP(
                            tensor=obb.tensor, offset=obb.offset + bp,
                            ap=[[4 * FH, 128], [56, 2], [112, NTH], [2, 28]])

                    q0 = tg_pool.tile([128, 2, NTH, 28], BF16, tag="q",
                                      name="q")
                    nc.gpsimd.tensor_tensor(out=q0[:], in0=sp[0][:],
                                            in1=sp[1][:], op=ADD)
                    nc.gpsimd.tensor_tensor(out=ob_view(0), in0=q0[:],
                                            in1=sp[2][:], op=ADD)
                    q1 = tg_pool.tile([128, 2, NTH, 28], BF16, tag="q",
                                      name="q")
                    nc.gpsimd.tensor_tensor(out=q1[:], in0=sp[1][:],
                                            in1=sp[2][:], op=SUB)
                    nc.gpsimd.tensor_tensor(out=ob_view(1), in0=q1[:],
                                            in1=sp[3][:], op=SUB)
                    nc.sync.dma_start(
                        out=out[im, mt][:, 28 * h * W: 28 * h * W + 4 * FH],
                        in_=ob[:])

            v00 = transform(0, 0)
            v01 = transform(0, 1)
            group(0, 0, v00)
            v10 = transform(1, 0)
            group(0, 1, v01)
            v11 = transform(1, 1)
            group(1, 0, v10)
            group(1, 1, v11)

    _split_multi_waits(nc)
    return nc


_NC_CACHE = None


def _get_nc():
    global _NC_CACHE
    if _NC_CACHE is None:
        _NC_CACHE = build_program()
    return _NC_CACHE


_G = np.array([[1, 0, 0], [.5, .5, .5], [.5, -.5, .5], [0, 0, 1]], np.float64)


def make_in_maps(x, centroids, labels, bias):
    """Shard full inputs into 8 per-core input maps (host-side gather,
    padding, and Winograd weight transform)."""
    x = np.ascontiguousarray(x, dtype=np.float32)
    centroids = np.ascontiguousarray(centroids, dtype=np.float32)
    labels = np.ascontiguousarray(labels, dtype=np.int64)
    bias = np.ascontiguousarray(bias, dtype=np.float32)

    xp = np.zeros((16, 256, HP, HP), dtype=ml_dtypes.bfloat16)
    xp[:, :, 1:1 + H, 1:1 + W] = x
    xp = xp.reshape(16, N_KT, 128, HWP)

    wq = centroids[labels]                       # [512, 256, 3, 3] f32
    U = np.einsum("ai,ocij,bj->aboc", _G, wq.astype(np.float64), _G)
    U[3] = -U[3]                                 # device row stage is pure add
    # -> lhsT layout [kt, cc, mt, p=4a+b, oo]
    U = U.reshape(16, N_MT, 128, N_KT, 128)      # [p, mt, oo, kt, cc]
    U = U.transpose(3, 4, 1, 0, 2)               # [kt, cc, mt, p, oo]
    U = np.ascontiguousarray(U.reshape(N_KT, 128, N_MT * N_PT * 128))
    U = U.astype(ml_dtypes.bfloat16)

    bias_l = np.ascontiguousarray(
        np.concatenate([bias[0], bias[1]]).reshape(N_MT, 128))

    in_maps = []
    for c in range(8):
        in_maps.append({
            "x": np.ascontiguousarray(xp[2 * c: 2 * c + 2]),
            "u": U,
            "bias": np.ascontiguousarray(bias_l),
        })
    return in_maps


def run(x, centroids, labels, bias, trace=False, trace_cores=None):
    nc = _get_nc()
    in_maps = make_in_maps(x, centroids, labels, bias)
    res = run_bass_kernel_spmd(nc, in_maps, list(range(8)), trace=trace,
                               trace_cores=trace_cores)
    out0 = np.empty((16, 256, H, W), dtype=np.float32)
    out1 = np.empty((16, 256, H, W), dtype=np.float32)
    for c in range(8):
        o = res.results[c]["out"].reshape(N_IMG, 512, H, W).astype(np.float32)
        out0[2 * c: 2 * c + 2] = o[:, :256]
        out1[2 * c: 2 * c + 2] = o[:, 256:]
    return (out0, out1), res


def kernel(x, centroids, labels, bias):
    (out0, out1), _ = run(x, centroids, labels, bias, trace=False)
    return (out0, out1)


# revision 35
# speedup vs baseline: 1.4372x; 1.1256x over previous
"""Trainium2 Bass kernel for nn_Conv2dShareQ (vq_codebook) — Winograd F(2x2,3x3).

Computation (see reference):
    wq = centroids[labels]            # [512, 256, 3, 3] fp32, 16-entry codebook
    out0 = conv2d(x, wq[:256], bias[0])   # NCHW, 3x3, stride 1, pad 1
    out1 = conv2d(x, wq[256:], bias[1])

Sharding: 8-way data-parallel over batch; each core runs 2 images through BOTH
convs (512 out channels = 4 m-tiles) so the transformed input V is shared.

Winograd F(2x2,3x3) cuts PE work 2.25x vs direct conv:
    Y = A^T [ (G w G^T) . (B^T d B) ] A   per 4x4 input tile, stride 2.
Host precomputes U = G wq G^T (a=3 slice negated so the device row stage is a
pure add) in matmul lhsT layout, plus zero-padded bf16 x.  Per (image, half):
  - input transform on DVE: 4+16 strided tensor_tensor ops -> V[16pt][128,392]
  - 8 matmuls per (mt, b): M_a = sum_kt U^T V into one 4-bank PSUM tile
  - row stage: ACT evicts M1,M2 (one op); GpSimd forms (M1+M2, M1-M2);
    DVE adds (M0, -M3) from PSUM -> S pair (bf16)
  - col stage on GpSimd: A-combos + bias via scalar_tensor_tensor, written
    strided into a row-major output tile; DMA out bf16 (host upcasts).
"""

import sys

for _p in ("/opt/trn_rl_repo", "/root/.axon_site/_ro/trn_rl_repo"):
    if _p not in sys.path:
        sys.path.append(_p)

import numpy as np
import ml_dtypes

import concourse.bass as bass
import concourse.mybir as mybir
from concourse.tile import TileContext, ScopedClock
from concourse.tile_scheduler import N_PROCS
from bass_rust import VectorClock
from concourse.bass_utils import run_bass_kernel_spmd

F32 = mybir.dt.float32
BF16 = mybir.dt.bfloat16
ADD = mybir.AluOpType.add
SUB = mybir.AluOpType.subtract
IDENT = mybir.ActivationFunctionType.Identity

N_IMG = 2               # 16 images / 8 cores
N_KT = 2                # 256 input channels / 128
N_MT = 4                # 512 output channels / 128 (both conv groups)
N_PT = 16               # winograd transform points
H = W = 56
HP = 58                 # padded
HWP = HP * HP
HW = H * W
NTH = 14                # winograd tile rows per h-half
FH = NTH * 28           # 392 free elems per (point, half)
N_WARM = 40


class SplitDrainTileContext(TileContext):
    """Tail drain split one proc per drain: this walrus build rejects CTRL
    instructions carrying more than one sem wait."""

    def _drain_and_barrier(self, tick_clock, wait_clock):
        gc = tick_clock.global_clock
        for p in range(N_PROCS):
            t = gc[p]
            if t <= 0:
                continue
            vec = [t if q == p else 0 for q in range(N_PROCS)]
            d = self.nc.sync.drain()
            wait_clock.add_sem_waits(d.ins, ScopedClock({None: VectorClock(vec)}))
        self.nc.all_engine_barrier()
        assert self.sems is not None
        popped = self.nc._tile_sem_poison_stack.pop()
        assert popped is self._sem_poison
        self.nc.clear_and_free_semaphores(list(self.sems.allocated().values()))
        self.nc.all_engine_barrier()


def _split_multi_waits(nc, limit=1):
    """This walrus build rejects instructions carrying more than one sem wait
    ("Too many sync wait commands").  Hoist excess waits onto wait-only
    EventSemaphore instructions inserted just before, on the same engine."""
    for f in nc.m.functions:
        for bb in f.blocks:
            out = []
            for ins in bb.instructions:
                si = ins.sync_info
                if si is not None and si.on_wait and len(si.on_wait) > limit:
                    waits = list(si.on_wait)
                    for w in waits[:-limit]:
                        es = mybir.InstEventSemaphore(
                            name=f"waitsplit_{nc.next_id()}", ins=[], outs=[])
                        es.engine = ins.engine
                        es.sync_info = mybir.SyncInfo(on_wait=[w], on_update=[])
                        out.append(es)
                    si.on_wait = waits[-limit:]
                out.append(ins)
            bb.instructions[:] = out


def build_program():
    nc = bass.Bass()

    x_in = nc.dram_tensor("x", [N_IMG, N_KT, 128, HWP], BF16,
                          kind="ExternalInput")
    u_in = nc.dram_tensor("u", [N_KT, 128, N_MT * N_PT * 128], BF16,
                          kind="ExternalInput")
    bias_in = nc.dram_tensor("bias", [N_MT, 128], F32, kind="ExternalInput")
    out = nc.dram_tensor("out", [N_IMG, N_MT, 128, HW], BF16,
                         kind="ExternalOutput")

    with SplitDrainTileContext(nc) as tc:
        with (
            tc.tile_pool(name="consts", bufs=1) as consts,
            tc.tile_pool(name="u", bufs=1) as u_pool,
            tc.tile_pool(name="xpad", bufs=1) as xpad_pool,
            tc.tile_pool(name="tT", bufs=1) as tT_pool,
            tc.tile_pool(name="v", bufs=3) as v_pool,
            tc.tile_pool(name="e", bufs=4) as e_pool,
            tc.tile_pool(name="tg", bufs=4) as tg_pool,
            tc.tile_pool(name="s", bufs=2) as s_pool,
            tc.tile_pool(name="ob", bufs=4) as ob_pool,
            tc.tile_pool(name="psum", bufs=2, space="PSUM") as psum_pool,
        ):
            u_sb = [u_pool.tile([128, N_MT * N_PT * 128], BF16, tag=f"u{kt}",
                                name=f"u{kt}") for kt in range(N_KT)]
            xpad = [[xpad_pool.tile([128, 2, HP, 29], BF16, tag=f"xp{im}_{kt}",
                                    name=f"xpad{im}_{kt}")
                     for kt in range(N_KT)] for im in range(N_IMG)]

            # ---- critical DMAs: mt0 weights for both kt + img0 x ----
            UC = N_PT * 128          # 2048 free elems per (kt, mt) chunk
            for kt in range(N_KT):
                nc.sync.dma_start(out=u_sb[kt][:, 0:UC], in_=u_in[kt][:, 0:UC])

            def load_x(im):
                # x is column-deinterleaved on host: [128, 2 par, 58, 29]
                for kt in range(N_KT):
                    xp = xpad[im][kt]
                    for par in range(2):
                        base = par * HP * 29
                        nc.sync.dma_start(
                            out=xp[:, par, 0:30, :],
                            in_=x_in[im, kt][:, base:base + 30 * 29])
                        nc.sync.dma_start(
                            out=xp[:, par, 30:HP, :],
                            in_=x_in[im, kt][:, base + 30 * 29:base + HP * 29])

            load_x(0)
            bias_sb = consts.tile([128, N_MT], F32)
            for mt in range(N_MT):
                nc.sync.dma_start(out=bias_sb[:, mt:mt + 1], in_=bias_in[mt, :])

            # ---- PE clock ramp on zeros while DMAs fly ----
            warm_sb = consts.tile([128, 256], BF16)
            nc.gpsimd.memset(warm_sb[:], 0.0)
            warm_ps = psum_pool.tile([128, 256], F32, tag="m")
            for _ in range(N_WARM):
                nc.tensor.matmul(warm_ps[:], warm_sb[:, :128], warm_sb[:],
                                 start=True, stop=True)

            # ---- remaining DMAs ----
            for mt in range(1, N_MT):
                for kt in range(N_KT):
                    nc.sync.dma_start(out=u_sb[kt][:, mt * UC:(mt + 1) * UC],
                                      in_=u_in[kt][:, mt * UC:(mt + 1) * UC])
            load_x(1)

            # ---- input transform: (im, h) -> V[kt] [128, 16, 14, 28] ----
            def transform(im, h):
                r0 = 28 * h
                vts = []
                for kt in range(N_KT):
                    xv = xpad[im][kt]          # [128, 2 par, 58, 29]
                    T = tT_pool.tile([128, 4, 2, NTH, 29], BF16, tag=f"T{kt}",
                                     name=f"T{kt}")
                    for a, (r1, r2, op) in enumerate(
                            [(0, 2, SUB), (1, 2, ADD), (2, 1, SUB),
                             (1, 3, SUB)]):
                        nc.vector.tensor_tensor(
                            out=T[:, a],
                            in0=xv[:, :, r0 + r1:min(r0 + r1 + 28, HP):2, :],
                            in1=xv[:, :, r0 + r2:min(r0 + r2 + 28, HP):2, :],
                            op=op)
                    v = v_pool.tile([128, N_PT, NTH, 28], BF16, tag=f"v{kt}",
                                    name=f"v{kt}")
                    # fused over a: v[:, b::4] = all four row-points of col b;
                    # deinterleaved cols make every combo a shifted window:
                    #   b0: ev[j]-ev[j+1]  b1: od[j]+ev[j+1]
                    #   b2: ev[j+1]-od[j]  b3: od[j]-od[j+1]
                    for b, (p0, j0, p1, j1, op) in enumerate(
                            [(0, 0, 0, 1, SUB), (1, 0, 0, 1, ADD),
                             (0, 1, 1, 0, SUB), (1, 0, 1, 1, SUB)]):
                        nc.vector.tensor_tensor(
                            out=v[:, b::4],
                            in0=T[:, :, p0, :, j0:j0 + 28],
                            in1=T[:, :, p1, :, j1:j1 + 28], op=op)
                    vts.append(v)
                return vts

            # ---- matmuls + output transform for one (im, h) ----
            def group(im, h, vts):
                for mt in range(N_MT):
                    ob = ob_pool.tile([128, NTH, 2, 28, 2], BF16, tag="ob",
                                      name="ob")
                    sp = []
                    for b in range(4):
                        M4 = psum_pool.tile([128, 4, 512], F32, tag="m",
                                            name="m")
                        for a in range(4):
                            p = 4 * a + b
                            off = (mt * N_PT + p) * 128
                            for kt in range(N_KT):
                                nc.tensor.matmul(
                                    M4[:, a, 0:FH],
                                    u_sb[kt][:, off:off + 128],
                                    vts[kt][:, p],
                                    start=(kt == 0), stop=(kt == N_KT - 1))
                        # bias enters once per output: e1 = M1 + b on the
                        # b==1 column makes t01 = M1+M2+b and t12 = M1-M2+b,
                        # and S_{a',1} appears with +1 in both Y columns
                        e12 = e_pool.tile([128, 2, FH], BF16, tag="e",
                                          name="e12")
                        if b == 1:
                            nc.scalar.activation(
                                out=e12[:, 0], in_=M4[:, 1, 0:FH], func=IDENT,
                                scale=1.0, bias=bias_sb[:, mt:mt + 1])
                            nc.scalar.activation(
                                out=e12[:, 1], in_=M4[:, 2, 0:FH], func=IDENT,
                                scale=1.0)
                        else:
                            nc.scalar.activation(
                                out=e12[:], in_=M4[:, 1:3, 0:FH], func=IDENT,
                                scale=1.0)
                        tp = tg_pool.tile([128, 2, FH], BF16, tag="tp",
                                          name="tp")
                        nc.vector.tensor_tensor(out=tp[:, 0], in0=e12[:, 0],
                                                in1=e12[:, 1], op=ADD)
                        nc.vector.tensor_tensor(out=tp[:, 1], in0=e12[:, 0],
                                                in1=e12[:, 1], op=SUB)
                        s2 = s_pool.tile([128, 2, NTH, 28], BF16,
                                         tag=f"s{b}", name=f"s{b}")
                        # S0 = M0 + (M1+M2); S1 = (M1-M2) + M3' (U a=3 negated)
                        nc.vector.tensor_tensor(out=s2[:],
                                                in0=M4[:, 0:4:3, 0:FH],
                                                in1=tp[:], op=ADD)
                        sp.append(s2)
                    # col stage fused over both output-row lanes; ob views
                    # iterate (lane, tile-row, tx) to match the S layout
                    obb = ob[:]

                    def ob_view(bp):
                        return bass.AP(
                            tensor=obb.tensor, offset=obb.offset + bp,
                            ap=[[4 * FH, 128], [56, 2], [112, NTH], [2, 28]])

                    q0 = tg_pool.tile([128, 2, NTH, 28], BF16, tag="q",
                                      name="q")
                    nc.gpsimd.tensor_tensor(out=q0[:], in0=sp[0][:],
                                            in1=sp[1][:], op=ADD)
                    nc.gpsimd.tensor_tensor(out=ob_view(0), in0=q0[:],
                                            in1=sp[2][:], op=ADD)
                    q1 = tg_pool.tile([128, 2, NTH, 28], BF16, tag="q",
                                      name="q")
                    nc.gpsimd.tensor_tensor(out=q1[:], in0=sp[1][:],
                                            in1=sp[2][:], op=SUB)
                    nc.gpsimd.tensor_tensor(out=ob_view(1), in0=q1[:],
                                            in1=sp[3][:], op=SUB)
                    nc.sync.dma_start(
                        out=out[im, mt][:, 28 * h * W: 28 * h * W + 4 * FH],
                        in_=ob[:])

            v00 = transform(0, 0)
            v01 = transform(0, 1)
            group(0, 0, v00)
            v10 = transform(1, 0)
            group(0, 1, v01)
            v11 = transform(1, 1)
            group(1, 0, v10)
            group(1, 1, v11)

    _split_multi_waits(nc)
    return nc


_NC_CACHE = None


def _get_nc():
    global _NC_CACHE
    if _NC_CACHE is None:
        _NC_CACHE = build_program()
    return _NC_CACHE


_G = np.array([[1, 0, 0], [.5, .5, .5], [.5, -.5, .5], [0, 0, 1]], np.float64)


def make_in_maps(x, centroids, labels, bias):
    """Shard full inputs into 8 per-core input maps (host-side gather,
    padding, and Winograd weight transform)."""
    x = np.ascontiguousarray(x, dtype=np.float32)
    centroids = np.ascontiguousarray(centroids, dtype=np.float32)
    labels = np.ascontiguousarray(labels, dtype=np.int64)
    bias = np.ascontiguousarray(bias, dtype=np.float32)

    xp = np.zeros((16, 256, HP, HP), dtype=ml_dtypes.bfloat16)
    xp[:, :, 1:1 + H, 1:1 + W] = x
    # deinterleave W into even/odd planes: [16, 256, 2, 58, 29]
    xp = np.stack([xp[:, :, :, 0::2], xp[:, :, :, 1::2]], axis=2)
    xp = np.ascontiguousarray(xp).reshape(16, N_KT, 128, HWP)

    wq = centroids[labels]                       # [512, 256, 3, 3] f32
    U = np.einsum("ai,ocij,bj->aboc", _G, wq.astype(np.float64), _G)
    U[3] = -U[3]                                 # device row stage is pure add
    # -> lhsT layout [kt, cc, mt, p=4a+b, oo]
    U = U.reshape(16, N_MT, 128, N_KT, 128)      # [p, mt, oo, kt, cc]
    U = U.transpose(3, 4, 1, 0, 2)               # [kt, cc, mt, p, oo]
    U = np.ascontiguousarray(U.reshape(N_KT, 128, N_MT * N_PT * 128))
    U = U.astype(ml_dtypes.bfloat16)

    bias_l = np.ascontiguousarray(
        np.concatenate([bias[0], bias[1]]).reshape(N_MT, 128))

    in_maps = []
    for c in range(8):
        in_maps.append({
            "x": np.ascontiguousarray(xp[2 * c: 2 * c + 2]),
            "u": U,
            "bias": np.ascontiguousarray(bias_l),
        })
    return in_maps


def run(x, centroids, labels, bias, trace=False, trace_cores=None):
    nc = _get_nc()
    in_maps = make_in_maps(x, centroids, labels, bias)
    res = run_bass_kernel_spmd(nc, in_maps, list(range(8)), trace=trace,
                               trace_cores=trace_cores)
    out0 = np.empty((16, 256, H, W), dtype=np.float32)
    out1 = np.empty((16, 256, H, W), dtype=np.float32)
    for c in range(8):
        o = res.results[c]["out"].reshape(N_IMG, 512, H, W).astype(np.float32)
        out0[2 * c: 2 * c + 2] = o[:, :256]
        out1[2 * c: 2 * c + 2] = o[:, 256:]
    return (out0, out1), res


def kernel(x, centroids, labels, bias):
    (out0, out1), _ = run(x, centroids, labels, bias, trace=False)
    return (out0, out1)


# revision 43
# speedup vs baseline: 2.0983x; 1.4600x over previous
"""Trainium2 Bass kernel for nn_Conv2dShareQ (vq_codebook) — Winograd F(2x2,3x3).

Computation (see reference):
    wq = centroids[labels]            # [512, 256, 3, 3] fp32, 16-entry codebook
    out0 = conv2d(x, wq[:256], bias[0])   # NCHW, 3x3, stride 1, pad 1
    out1 = conv2d(x, wq[256:], bias[1])

Sharding: 8-way data-parallel over batch; each core runs 2 images through BOTH
convs (512 out channels = 4 m-tiles) so the transformed input V is shared.

Winograd F(2x2,3x3) cuts PE work 2.25x vs direct conv:
    Y = A^T [ (G w G^T) . (B^T d B) ] A   per 4x4 input tile, stride 2.
Host precomputes U = G wq G^T (a=3 slice negated so the device row stage is a
pure add) in matmul lhsT layout, plus zero-padded bf16 x.  Per (image, half):
  - input transform on DVE: 4+16 strided tensor_tensor ops -> V[16pt][128,392]
  - 8 matmuls per (mt, b): M_a = sum_kt U^T V into one 4-bank PSUM tile
  - row stage: ACT evicts M1,M2 (one op); GpSimd forms (M1+M2, M1-M2);
    DVE adds (M0, -M3) from PSUM -> S pair (bf16)
  - col stage on GpSimd: A-combos + bias via scalar_tensor_tensor, written
    strided into a row-major output tile; DMA out bf16 (host upcasts).
"""

import sys

for _p in ("/opt/trn_rl_repo", "/root/.axon_site/_ro/trn_rl_repo"):
    if _p not in sys.path:
        sys.path.append(_p)

import numpy as np
import ml_dtypes

import concourse.bass as bass
import concourse.mybir as mybir
from concourse.tile import TileContext, ScopedClock
from concourse.tile_scheduler import N_PROCS
from bass_rust import VectorClock
from concourse.bass_utils import run_bass_kernel_spmd

F32 = mybir.dt.float32
BF16 = mybir.dt.bfloat16
ADD = mybir.AluOpType.add
SUB = mybir.AluOpType.subtract
IDENT = mybir.ActivationFunctionType.Identity

N_IMG = 2               # 16 images / 8 cores
N_KT = 2                # 256 input channels / 128
N_MT = 4                # 512 output channels / 128 (both conv groups)
N_PT = 16               # winograd transform points
H = W = 56
HP = 58                 # padded
HWP = HP * HP
HW = H * W
NTH = 14                # winograd tile rows per h-half
FH = NTH * 28           # 392 free elems per (point, half)
N_WARM = 40


class SplitDrainTileContext(TileContext):
    """Tail drain split one proc per drain: this walrus build rejects CTRL
    instructions carrying more than one sem wait."""

    def _drain_and_barrier(self, tick_clock, wait_clock):
        gc = tick_clock.global_clock
        for p in range(N_PROCS):
            t = gc[p]
            if t <= 0:
                continue
            vec = [t if q == p else 0 for q in range(N_PROCS)]
            d = self.nc.sync.drain()
            wait_clock.add_sem_waits(d.ins, ScopedClock({None: VectorClock(vec)}))
        self.nc.all_engine_barrier()
        assert self.sems is not None
        popped = self.nc._tile_sem_poison_stack.pop()
        assert popped is self._sem_poison
        self.nc.clear_and_free_semaphores(list(self.sems.allocated().values()))
        self.nc.all_engine_barrier()


def _split_multi_waits(nc, limit=1):
    """This walrus build rejects instructions carrying more than one sem wait
    ("Too many sync wait commands").  Hoist excess waits onto wait-only
    EventSemaphore instructions inserted just before, on the same engine."""
    for f in nc.m.functions:
        for bb in f.blocks:
            out = []
            for ins in bb.instructions:
                si = ins.sync_info
                if si is not None and si.on_wait and len(si.on_wait) > limit:
                    waits = list(si.on_wait)
                    for w in waits[:-limit]:
                        es = mybir.InstEventSemaphore(
                            name=f"waitsplit_{nc.next_id()}", ins=[], outs=[])
                        es.engine = ins.engine
                        es.sync_info = mybir.SyncInfo(on_wait=[w], on_update=[])
                        out.append(es)
                    si.on_wait = waits[-limit:]
                out.append(ins)
            bb.instructions[:] = out


def build_program():
    nc = bass.Bass()

    x_in = nc.dram_tensor("x", [N_IMG, N_KT, 128, HWP], BF16,
                          kind="ExternalInput")
    # U'' = row-transform folded into PE: per (mt, b) two S-slots of
    # three signed U points each -> 4*4*2*3*128 = 12288 free elems per kt
    u_in = nc.dram_tensor("u", [N_KT, 128, N_MT * 4 * 2 * 3 * 128], BF16,
                          kind="ExternalInput")
    bias_in = nc.dram_tensor("bias", [N_MT, 128], F32, kind="ExternalInput")
    out = nc.dram_tensor("out", [N_IMG, N_MT, 128, HW], BF16,
                         kind="ExternalOutput")

    with SplitDrainTileContext(nc) as tc:
        with (
            tc.tile_pool(name="consts", bufs=1) as consts,
            tc.tile_pool(name="u", bufs=1) as u_pool,
            tc.tile_pool(name="xpad", bufs=1) as xpad_pool,
            tc.tile_pool(name="tT", bufs=1) as tT_pool,
            tc.tile_pool(name="v", bufs=3) as v_pool,
            tc.tile_pool(name="tg", bufs=4) as tg_pool,
            tc.tile_pool(name="s", bufs=2) as s_pool,
            tc.tile_pool(name="ob", bufs=4) as ob_pool,
            tc.tile_pool(name="psum", bufs=4, space="PSUM") as psum_pool,
        ):
            u_sb = [u_pool.tile([128, N_MT * 4 * 2 * 3 * 128], BF16,
                                tag=f"u{kt}", name=f"u{kt}")
                    for kt in range(N_KT)]
            xpad = [[xpad_pool.tile([128, 2, HP, 29], BF16, tag=f"xp{im}_{kt}",
                                    name=f"xpad{im}_{kt}")
                     for kt in range(N_KT)] for im in range(N_IMG)]

            # ---- critical DMAs: mt0 weights for both kt + img0 x ----
            UC = 4 * 2 * 3 * 128     # 3072 free elems per (kt, mt) chunk
            for kt in range(N_KT):
                nc.sync.dma_start(out=u_sb[kt][:, 0:UC], in_=u_in[kt][:, 0:UC])

            def load_x(im):
                # x is column-deinterleaved on host: [128, 2 par, 58, 29]
                for kt in range(N_KT):
                    xp = xpad[im][kt]
                    for par in range(2):
                        base = par * HP * 29
                        nc.sync.dma_start(
                            out=xp[:, par, 0:30, :],
                            in_=x_in[im, kt][:, base:base + 30 * 29])
                        nc.sync.dma_start(
                            out=xp[:, par, 30:HP, :],
                            in_=x_in[im, kt][:, base + 30 * 29:base + HP * 29])

            load_x(0)
            bias_sb = consts.tile([128, N_MT], F32)
            for mt in range(N_MT):
                nc.sync.dma_start(out=bias_sb[:, mt:mt + 1], in_=bias_in[mt, :])

            # ---- PE clock ramp on zeros while DMAs fly ----
            warm_sb = consts.tile([128, 256], BF16)
            nc.gpsimd.memset(warm_sb[:], 0.0)
            warm_ps = psum_pool.tile([128, 256], F32, tag="m")
            for _ in range(N_WARM):
                nc.tensor.matmul(warm_ps[:], warm_sb[:, :128], warm_sb[:],
                                 start=True, stop=True)

            # ---- remaining DMAs ----
            for mt in range(1, N_MT):
                for kt in range(N_KT):
                    nc.sync.dma_start(out=u_sb[kt][:, mt * UC:(mt + 1) * UC],
                                      in_=u_in[kt][:, mt * UC:(mt + 1) * UC])
            load_x(1)

            # ---- input transform: (im, h) -> V[kt] [128, 16, 14, 28] ----
            def transform(im, h):
                r0 = 28 * h
                vts = []
                for kt in range(N_KT):
                    xv = xpad[im][kt]          # [128, 2 par, 58, 29]
                    T = tT_pool.tile([128, 4, 2, NTH, 29], BF16, tag=f"T{kt}",
                                     name=f"T{kt}")
                    for a, (r1, r2, op) in enumerate(
                            [(0, 2, SUB), (1, 2, ADD), (2, 1, SUB),
                             (1, 3, SUB)]):
                        nc.vector.tensor_tensor(
                            out=T[:, a],
                            in0=xv[:, :, r0 + r1:min(r0 + r1 + 28, HP):2, :],
                            in1=xv[:, :, r0 + r2:min(r0 + r2 + 28, HP):2, :],
                            op=op)
                    v = v_pool.tile([128, N_PT, NTH, 28], BF16, tag=f"v{kt}",
                                    name=f"v{kt}")
                    # fused over a: v[:, b::4] = all four row-points of col b;
                    # deinterleaved cols make every combo a shifted window:
                    #   b0: ev[j]-ev[j+1]  b1: od[j]+ev[j+1]
                    #   b2: ev[j+1]-od[j]  b3: od[j]-od[j+1]
                    for b, (p0, j0, p1, j1, op) in enumerate(
                            [(0, 0, 0, 1, SUB), (1, 0, 0, 1, ADD),
                             (0, 1, 1, 0, SUB), (1, 0, 1, 1, SUB)]):
                        nc.vector.tensor_tensor(
                            out=v[:, b::4],
                            in0=T[:, :, p0, :, j0:j0 + 28],
                            in1=T[:, :, p1, :, j1:j1 + 28], op=op)
                    vts.append(v)
                return vts

            # ---- matmuls + output transform for one (im, h) ----
            # S-slot a-points and signs are baked into U'' on the host:
            #   slot 0: +U0 +U1 +U2   slot 1: +U1 -U2 -U3
            A_OF = [[0, 1, 2], [1, 2, 3]]

            def group(im, h, vts):
                for mt in range(N_MT):
                    ob = ob_pool.tile([128, NTH, 2, 28, 2], BF16, tag="ob",
                                      name="ob")
                    sp = []
                    for b in range(4):
                        PS = psum_pool.tile([128, 2, 512], F32, tag="m",
                                            name="m")
                        for s in range(2):
                            for j in range(3):
                                p = 4 * A_OF[s][j] + b
                                off = ((((mt * 4 + b) * 2 + s) * 3 + j)) * 128
                                for kt in range(N_KT):
                                    nc.tensor.matmul(
                                        PS[:, s, 0:FH],
                                        u_sb[kt][:, off:off + 128],
                                        vts[kt][:, p],
                                        start=(j == 0 and kt == 0),
                                        stop=(j == 2 and kt == N_KT - 1))
                        s2 = s_pool.tile([128, 2, NTH, 28], BF16,
                                         tag=f"s{b}", name=f"s{b}")
                        # bias once per output: both S lanes of the b==1
                        # column appear with +1 in each Y column combo
                        nc.scalar.activation(
                            out=s2[:], in_=PS[:, :, 0:FH], func=IDENT,
                            scale=1.0,
                            bias=(bias_sb[:, mt:mt + 1] if b == 1 else 0.0))
                        sp.append(s2)
                    # col stage fused over both output-row lanes; ob views
                    # iterate (lane, tile-row, tx) to match the S layout
                    obb = ob[:]

                    def ob_view(bp):
                        return bass.AP(
                            tensor=obb.tensor, offset=obb.offset + bp,
                            ap=[[4 * FH, 128], [56, 2], [112, NTH], [2, 28]])

                    q0 = tg_pool.tile([128, 2, NTH, 28], BF16, tag="q",
                                      name="q")
                    nc.vector.tensor_tensor(out=q0[:], in0=sp[0][:],
                                            in1=sp[1][:], op=ADD)
                    nc.vector.tensor_tensor(out=ob_view(0), in0=q0[:],
                                            in1=sp[2][:], op=ADD)
                    q1 = tg_pool.tile([128, 2, NTH, 28], BF16, tag="q",
                                      name="q")
                    nc.gpsimd.tensor_tensor(out=q1[:], in0=sp[1][:],
                                            in1=sp[2][:], op=SUB)
                    nc.gpsimd.tensor_tensor(out=ob_view(1), in0=q1[:],
                                            in1=sp[3][:], op=SUB)
                    nc.sync.dma_start(
                        out=out[im, mt][:, 28 * h * W: 28 * h * W + 4 * FH],
                        in_=ob[:])

            v00 = transform(0, 0)
            v01 = transform(0, 1)
            group(0, 0, v00)
            v10 = transform(1, 0)
            group(0, 1, v01)
            v11 = transform(1, 1)
            group(1, 0, v10)
            group(1, 1, v11)

    _split_multi_waits(nc)
    return nc


_NC_CACHE = None


def _get_nc():
    global _NC_CACHE
    if _NC_CACHE is None:
        _NC_CACHE = build_program()
    return _NC_CACHE


_G = np.array([[1, 0, 0], [.5, .5, .5], [.5, -.5, .5], [0, 0, 1]], np.float64)


def make_in_maps(x, centroids, labels, bias):
    """Shard full inputs into 8 per-core input maps (host-side gather,
    padding, and Winograd weight transform)."""
    x = np.ascontiguousarray(x, dtype=np.float32)
    centroids = np.ascontiguousarray(centroids, dtype=np.float32)
    labels = np.ascontiguousarray(labels, dtype=np.int64)
    bias = np.ascontiguousarray(bias, dtype=np.float32)

    xp = np.zeros((16, 256, HP, HP), dtype=ml_dtypes.bfloat16)
    xp[:, :, 1:1 + H, 1:1 + W] = x
    # deinterleave W into even/odd planes: [16, 256, 2, 58, 29]
    xp = np.stack([xp[:, :, :, 0::2], xp[:, :, :, 1::2]], axis=2)
    xp = np.ascontiguousarray(xp).reshape(16, N_KT, 128, HWP)

    wq = centroids[labels]                       # [512, 256, 3, 3] f32
    U = np.einsum("ai,ocij,bj->aboc", _G, wq.astype(np.float64), _G)
    # fold the A^T row combos into the weights: per (b, slot) three signed
    # points; slot0 = +U0 +U1 +U2, slot1 = +U1 -U2 -U3
    U2 = np.empty((4, 2, 3, 512, 256), np.float64)
    for s, (alist, signs) in enumerate(
            [((0, 1, 2), (1, 1, 1)), ((1, 2, 3), (1, -1, -1))]):
        for j, (a, sg) in enumerate(zip(alist, signs)):
            U2[:, s, j] = sg * U[a]              # [b, s, j, oc, ic]
    # -> lhsT layout [kt, cc, mt, b, s, j, oo]
    U2 = U2.reshape(4, 2, 3, N_MT, 128, N_KT, 128)
    U2 = U2.transpose(5, 6, 3, 0, 1, 2, 4)       # [kt, cc, mt, b, s, j, oo]
    U2 = np.ascontiguousarray(U2.reshape(N_KT, 128, N_MT * 4 * 2 * 3 * 128))
    U2 = U2.astype(ml_dtypes.bfloat16)

    bias_l = np.ascontiguousarray(
        np.concatenate([bias[0], bias[1]]).reshape(N_MT, 128))

    in_maps = []
    for c in range(8):
        in_maps.append({
            "x": np.ascontiguousarray(xp[2 * c: 2 * c + 2]),
            "u": U2,
            "bias": np.ascontiguousarray(bias_l),
        })
    return in_maps


def run(x, centroids, labels, bias, trace=False, trace_cores=None):
    nc = _get_nc()
    in_maps = make_in_maps(x, centroids, labels, bias)
    res = run_bass_kernel_spmd(nc, in_maps, list(range(8)), trace=trace,
                               trace_cores=trace_cores)
    out0 = np.empty((16, 256, H, W), dtype=np.float32)
    out1 = np.empty((16, 256, H, W), dtype=np.float32)
    for c in range(8):
        o = res.results[c]["out"].reshape(N_IMG, 512, H, W).astype(np.float32)
        out0[2 * c: 2 * c + 2] = o[:, :256]
        out1[2 * c: 2 * c + 2] = o[:, 256:]
    return (out0, out1), res


def kernel(x, centroids, labels, bias):
    (out0, out1), _ = run(x, centroids, labels, bias, trace=False)
    return (out0, out1)


# revision 47
# speedup vs baseline: 2.1480x; 1.0237x over previous
"""Trainium2 Bass kernel for nn_Conv2dShareQ (vq_codebook) — Winograd F(2x2,3x3).

Computation (see reference):
    wq = centroids[labels]            # [512, 256, 3, 3] fp32, 16-entry codebook
    out0 = conv2d(x, wq[:256], bias[0])   # NCHW, 3x3, stride 1, pad 1
    out1 = conv2d(x, wq[256:], bias[1])

Sharding: 8-way data-parallel over batch; each core runs 2 images through BOTH
convs (512 out channels = 4 m-tiles) so the transformed input V is shared.

Winograd F(2x2,3x3) cuts PE work 2.25x vs direct conv:
    Y = A^T [ (G w G^T) . (B^T d B) ] A   per 4x4 input tile, stride 2.
Host precomputes U = G wq G^T (a=3 slice negated so the device row stage is a
pure add) in matmul lhsT layout, plus zero-padded bf16 x.  Per (image, half):
  - input transform on DVE: 4+16 strided tensor_tensor ops -> V[16pt][128,392]
  - 8 matmuls per (mt, b): M_a = sum_kt U^T V into one 4-bank PSUM tile
  - row stage: ACT evicts M1,M2 (one op); GpSimd forms (M1+M2, M1-M2);
    DVE adds (M0, -M3) from PSUM -> S pair (bf16)
  - col stage on GpSimd: A-combos + bias via scalar_tensor_tensor, written
    strided into a row-major output tile; DMA out bf16 (host upcasts).
"""

import sys

for _p in ("/opt/trn_rl_repo", "/root/.axon_site/_ro/trn_rl_repo"):
    if _p not in sys.path:
        sys.path.append(_p)

import numpy as np
import ml_dtypes

import concourse.bass as bass
import concourse.mybir as mybir
from concourse.tile import TileContext, ScopedClock
from concourse.tile_scheduler import N_PROCS
from bass_rust import VectorClock
from concourse.bass_utils import run_bass_kernel_spmd

F32 = mybir.dt.float32
BF16 = mybir.dt.bfloat16
ADD = mybir.AluOpType.add
SUB = mybir.AluOpType.subtract
IDENT = mybir.ActivationFunctionType.Identity

N_IMG = 2               # 16 images / 8 cores
N_KT = 2                # 256 input channels / 128
N_MT = 4                # 512 output channels / 128 (both conv groups)
N_PT = 16               # winograd transform points
H = W = 56
HP = 58                 # padded
HWP = HP * HP
HW = H * W
NTH = 14                # winograd tile rows per h-half
FH = NTH * 28           # 392 free elems per (point, half)
N_WARM = 40


class SplitDrainTileContext(TileContext):
    """Tail drain split one proc per drain: this walrus build rejects CTRL
    instructions carrying more than one sem wait."""

    def _drain_and_barrier(self, tick_clock, wait_clock):
        gc = tick_clock.global_clock
        for p in range(N_PROCS):
            t = gc[p]
            if t <= 0:
                continue
            vec = [t if q == p else 0 for q in range(N_PROCS)]
            d = self.nc.sync.drain()
            wait_clock.add_sem_waits(d.ins, ScopedClock({None: VectorClock(vec)}))
        self.nc.all_engine_barrier()
        assert self.sems is not None
        popped = self.nc._tile_sem_poison_stack.pop()
        assert popped is self._sem_poison
        self.nc.clear_and_free_semaphores(list(self.sems.allocated().values()))
        self.nc.all_engine_barrier()


def _split_multi_waits(nc, limit=1):
    """This walrus build rejects instructions carrying more than one sem wait
    ("Too many sync wait commands").  Hoist excess waits onto wait-only
    EventSemaphore instructions inserted just before, on the same engine."""
    for f in nc.m.functions:
        for bb in f.blocks:
            out = []
            for ins in bb.instructions:
                si = ins.sync_info
                if si is not None and si.on_wait and len(si.on_wait) > limit:
                    waits = list(si.on_wait)
                    for w in waits[:-limit]:
                        es = mybir.InstEventSemaphore(
                            name=f"waitsplit_{nc.next_id()}", ins=[], outs=[])
                        es.engine = ins.engine
                        es.sync_info = mybir.SyncInfo(on_wait=[w], on_update=[])
                        out.append(es)
                    si.on_wait = waits[-limit:]
                out.append(ins)
            bb.instructions[:] = out


def build_program():
    nc = bass.Bass()

    x_in = nc.dram_tensor("x", [N_IMG, N_KT, 128, HWP], BF16,
                          kind="ExternalInput")
    # U'' = row-transform folded into PE: per (mt, b) two S-slots of
    # three signed U points each -> 4*4*2*3*128 = 12288 free elems per kt
    u_in = nc.dram_tensor("u", [N_KT, 128, N_MT * 4 * 2 * 3 * 128], BF16,
                          kind="ExternalInput")
    bias_in = nc.dram_tensor("bias", [N_MT, 128], F32, kind="ExternalInput")
    out = nc.dram_tensor("out", [N_IMG, N_MT, 128, HW], BF16,
                         kind="ExternalOutput")

    with SplitDrainTileContext(nc) as tc:
        with (
            tc.tile_pool(name="consts", bufs=1) as consts,
            tc.tile_pool(name="u", bufs=1) as u_pool,
            tc.tile_pool(name="xpad", bufs=1) as xpad_pool,
            tc.tile_pool(name="tT", bufs=1) as tT_pool,
            tc.tile_pool(name="v", bufs=3) as v_pool,
            tc.tile_pool(name="tg", bufs=4) as tg_pool,
            tc.tile_pool(name="s", bufs=2) as s_pool,
            tc.tile_pool(name="ob", bufs=4) as ob_pool,
            tc.tile_pool(name="psum", bufs=4, space="PSUM") as psum_pool,
        ):
            u_sb = [u_pool.tile([128, N_MT * 4 * 2 * 3 * 128], BF16,
                                tag=f"u{kt}", name=f"u{kt}")
                    for kt in range(N_KT)]
            xpad = [[xpad_pool.tile([128, 2, HP, 29], BF16, tag=f"xp{im}_{kt}",
                                    name=f"xpad{im}_{kt}")
                     for kt in range(N_KT)] for im in range(N_IMG)]

            # ---- critical DMAs: mt0 weights for both kt + img0 x ----
            UC = 4 * 2 * 3 * 128     # 3072 free elems per (kt, mt) chunk
            for kt in range(N_KT):
                nc.sync.dma_start(out=u_sb[kt][:, 0:UC], in_=u_in[kt][:, 0:UC])

            def load_x(im):
                # x is column-deinterleaved on host: [128, 2 par, 58, 29];
                # rows 0-29 (both parities) first — the h=0 transform's input
                for kt in range(N_KT):
                    xp = xpad[im][kt]
                    for par in range(2):
                        base = par * HP * 29
                        nc.sync.dma_start(
                            out=xp[:, par, 0:30, :],
                            in_=x_in[im, kt][:, base:base + 30 * 29])
                for kt in range(N_KT):
                    xp = xpad[im][kt]
                    for par in range(2):
                        base = par * HP * 29
                        nc.sync.dma_start(
                            out=xp[:, par, 30:HP, :],
                            in_=x_in[im, kt][:, base + 30 * 29:base + HP * 29])

            load_x(0)
            bias_sb = consts.tile([128, N_MT], F32)
            for mt in range(N_MT):
                nc.sync.dma_start(out=bias_sb[:, mt:mt + 1], in_=bias_in[mt, :])

            # ---- PE clock ramp on zeros while DMAs fly ----
            warm_sb = consts.tile([128, 256], BF16)
            nc.gpsimd.memset(warm_sb[:], 0.0)
            warm_ps = psum_pool.tile([128, 256], F32, tag="m")
            for _ in range(N_WARM):
                nc.tensor.matmul(warm_ps[:], warm_sb[:, :128], warm_sb[:],
                                 start=True, stop=True)

            # ---- remaining DMAs ----
            for mt in range(1, N_MT):
                for kt in range(N_KT):
                    nc.sync.dma_start(out=u_sb[kt][:, mt * UC:(mt + 1) * UC],
                                      in_=u_in[kt][:, mt * UC:(mt + 1) * UC])
            load_x(1)

            # ---- input transform: (im, h) -> V[kt] [128, 16, 14, 28] ----
            def transform(im, h):
                r0 = 28 * h
                vts = []
                for kt in range(N_KT):
                    xv = xpad[im][kt]          # [128, 2 par, 58, 29]
                    T = tT_pool.tile([128, 4, 2, NTH, 29], BF16, tag=f"T{kt}",
                                     name=f"T{kt}")
                    for a, (r1, r2, op) in enumerate(
                            [(0, 2, SUB), (1, 2, ADD), (2, 1, SUB),
                             (1, 3, SUB)]):
                        nc.vector.tensor_tensor(
                            out=T[:, a],
                            in0=xv[:, :, r0 + r1:min(r0 + r1 + 28, HP):2, :],
                            in1=xv[:, :, r0 + r2:min(r0 + r2 + 28, HP):2, :],
                            op=op)
                    v = v_pool.tile([128, N_PT, NTH, 28], BF16, tag=f"v{kt}",
                                    name=f"v{kt}")
                    # fused over a: v[:, b::4] = all four row-points of col b;
                    # deinterleaved cols make every combo a shifted window:
                    #   b0: ev[j]-ev[j+1]  b1: od[j]+ev[j+1]
                    #   b2: ev[j+1]-od[j]  b3: od[j]-od[j+1]
                    for b, (p0, j0, p1, j1, op) in enumerate(
                            [(0, 0, 0, 1, SUB), (1, 0, 0, 1, ADD),
                             (0, 1, 1, 0, SUB), (1, 0, 1, 1, SUB)]):
                        nc.vector.tensor_tensor(
                            out=v[:, b::4],
                            in0=T[:, :, p0, :, j0:j0 + 28],
                            in1=T[:, :, p1, :, j1:j1 + 28], op=op)
                    vts.append(v)
                return vts

            # ---- matmuls + output transform for one (im, h) ----
            # S-slot a-points and signs are baked into U'' on the host:
            #   slot 0: +U0 +U1 +U2   slot 1: +U1 -U2 -U3
            A_OF = [[0, 1, 2], [1, 2, 3]]

            def group(im, h, vts):
                for mt in range(N_MT):
                    # output stored bp-deinterleaved: [bp, t, ap, tx];
                    # host re-interleaves the even/odd output columns
                    ob = ob_pool.tile([128, 2, NTH, 2, 28], BF16, tag="ob",
                                      name="ob")
                    sp = []
                    for b in range(4):
                        PS = psum_pool.tile([128, 2, 512], F32, tag="m",
                                            name="m")
                        for s in range(2):
                            for j in range(3):
                                p = 4 * A_OF[s][j] + b
                                off = ((((mt * 4 + b) * 2 + s) * 3 + j)) * 128
                                for kt in range(N_KT):
                                    nc.tensor.matmul(
                                        PS[:, s, 0:FH],
                                        u_sb[kt][:, off:off + 128],
                                        vts[kt][:, p],
                                        start=(j == 0 and kt == 0),
                                        stop=(j == 2 and kt == N_KT - 1))
                        s2 = s_pool.tile([128, 2, NTH, 28], BF16,
                                         tag=f"s{b}", name=f"s{b}")
                        # bias once per output: both S lanes of the b==1
                        # column appear with +1 in each Y column combo
                        nc.scalar.activation(
                            out=s2[:], in_=PS[:, :, 0:FH], func=IDENT,
                            scale=1.0,
                            bias=(bias_sb[:, mt:mt + 1] if b == 1 else 0.0))
                        sp.append(s2)
                    # col stage fused over both output-row lanes; ob views
                    # iterate (lane, tile-row, tx) to match the S layout
                    obb = ob[:]

                    def ob_view(bp):
                        return bass.AP(
                            tensor=obb.tensor, offset=obb.offset + bp * 2 * FH,
                            ap=[[4 * FH, 128], [28, 2], [56, NTH], [1, 28]])

                    q0 = tg_pool.tile([128, 2, NTH, 28], BF16, tag="q",
                                      name="q")
                    nc.vector.tensor_tensor(out=q0[:], in0=sp[0][:],
                                            in1=sp[1][:], op=ADD)
                    nc.vector.tensor_tensor(out=ob_view(0), in0=q0[:],
                                            in1=sp[2][:], op=ADD)
                    q1 = tg_pool.tile([128, 2, NTH, 28], BF16, tag="q",
                                      name="q")
                    nc.gpsimd.tensor_tensor(out=q1[:], in0=sp[1][:],
                                            in1=sp[2][:], op=SUB)
                    nc.gpsimd.tensor_tensor(out=ob_view(1), in0=q1[:],
                                            in1=sp[3][:], op=SUB)
                    nc.sync.dma_start(
                        out=out[im, mt][:, 28 * h * W: 28 * h * W + 4 * FH],
                        in_=ob[:])

            v00 = transform(0, 0)
            v01 = transform(0, 1)
            group(0, 0, v00)
            v10 = transform(1, 0)
            group(0, 1, v01)
            v11 = transform(1, 1)
            group(1, 0, v10)
            group(1, 1, v11)

    _split_multi_waits(nc)
    return nc


_NC_CACHE = None


def _get_nc():
    global _NC_CACHE
    if _NC_CACHE is None:
        _NC_CACHE = build_program()
    return _NC_CACHE


_G = np.array([[1, 0, 0], [.5, .5, .5], [.5, -.5, .5], [0, 0, 1]], np.float64)


def make_in_maps(x, centroids, labels, bias):
    """Shard full inputs into 8 per-core input maps (host-side gather,
    padding, and Winograd weight transform)."""
    x = np.ascontiguousarray(x, dtype=np.float32)
    centroids = np.ascontiguousarray(centroids, dtype=np.float32)
    labels = np.ascontiguousarray(labels, dtype=np.int64)
    bias = np.ascontiguousarray(bias, dtype=np.float32)

    xp = np.zeros((16, 256, HP, HP), dtype=ml_dtypes.bfloat16)
    xp[:, :, 1:1 + H, 1:1 + W] = x
    # deinterleave W into even/odd planes: [16, 256, 2, 58, 29]
    xp = np.stack([xp[:, :, :, 0::2], xp[:, :, :, 1::2]], axis=2)
    xp = np.ascontiguousarray(xp).reshape(16, N_KT, 128, HWP)

    wq = centroids[labels]                       # [512, 256, 3, 3] f32
    U = np.einsum("ai,ocij,bj->aboc", _G, wq.astype(np.float64), _G)
    # fold the A^T row combos into the weights: per (b, slot) three signed
    # points; slot0 = +U0 +U1 +U2, slot1 = +U1 -U2 -U3
    U2 = np.empty((4, 2, 3, 512, 256), np.float64)
    for s, (alist, signs) in enumerate(
            [((0, 1, 2), (1, 1, 1)), ((1, 2, 3), (1, -1, -1))]):
        for j, (a, sg) in enumerate(zip(alist, signs)):
            U2[:, s, j] = sg * U[a]              # [b, s, j, oc, ic]
    # -> lhsT layout [kt, cc, mt, b, s, j, oo]
    U2 = U2.reshape(4, 2, 3, N_MT, 128, N_KT, 128)
    U2 = U2.transpose(5, 6, 3, 0, 1, 2, 4)       # [kt, cc, mt, b, s, j, oo]
    U2 = np.ascontiguousarray(U2.reshape(N_KT, 128, N_MT * 4 * 2 * 3 * 128))
    U2 = U2.astype(ml_dtypes.bfloat16)

    bias_l = np.ascontiguousarray(
        np.concatenate([bias[0], bias[1]]).reshape(N_MT, 128))

    in_maps = []
    for c in range(8):
        in_maps.append({
            "x": np.ascontiguousarray(xp[2 * c: 2 * c + 2]),
            "u": U2,
            "bias": np.ascontiguousarray(bias_l),
        })
    return in_maps


def run(x, centroids, labels, bias, trace=False, trace_cores=None):
    nc = _get_nc()
    in_maps = make_in_maps(x, centroids, labels, bias)
    res = run_bass_kernel_spmd(nc, in_maps, list(range(8)), trace=trace,
                               trace_cores=trace_cores)
    out0 = np.empty((16, 256, H, W), dtype=np.float32)
    out1 = np.empty((16, 256, H, W), dtype=np.float32)
    for c in range(8):
        o = res.results[c]["out"].astype(np.float32)
        # [im, mt, oo, h, bp, r, c] -> rows (h, 2r+ap baked in r), cols (c, bp)
        o = o.reshape(N_IMG, N_MT, 128, 2, 2, 28, 28)
        o = o.transpose(0, 1, 2, 3, 5, 6, 4)
        o = o.reshape(N_IMG, 512, H, W)
        out0[2 * c: 2 * c + 2] = o[:, :256]
        out1[2 * c: 2 * c + 2] = o[:, 256:]
    return (out0, out1), res


def kernel(x, centroids, labels, bias):
    (out0, out1), _ = run(x, centroids, labels, bias, trace=False)
    return (out0, out1)
